# revision 24
# baseline (speedup 1.0000x reference)
"""DigitCapsules dynamic-routing kernel for 8 TRN2 NeuronCores.

Strategy (hardcoded for B=128, R=2048, O=16, D=16, C=16, 3 routing iters):
  - Shard R across the 8 cores (256 routes/core); x replicated.
  - u_hat = x @ W on TensorE (K=16 matmuls packed 4x via row tile_position),
    kept SBUF-resident as f16 [b=128 partitions, (o, c, r)], r innermost.
  - Stationaries are loaded once per run with standalone ldweights();
    the matmuls set InstMatmult.ldweights=False (verified on HW), so a
    matmul costs only its column stream.
  - All routing reductions run on TensorE: an identity-stationary matmul
    whose output AP is broadcast (stride 0) over the reduced dim makes PSUM
    accumulate the slices per element (verified on HW), so
      s = sum_o c*u, a = sum_c u*v, ns = sum_c s^2
    cost one PSUM-bank-sized matmul group streaming the product tile. DVE
    only does the elementwise products (f16, 2x mode); ScalarE drains PSUM
    and computes exp/square/sqrt (activation tables are preloaded once).
  - Iteration 0 uses uniform c_ij: s0 = x @ Wbar with Wbar = sum_o W / R
    precomputed on host.
  - Softmax state is multiplicative: q <- q * exp(a_psum); softmax over
    global R only needs the denominator AllReduce (8KB) per iteration;
    normalization is scale-invariant so q is rescaled in place.
  - The warmup AllReduce doubles as a cross-core barrier (its output DMA
    gates the later weight loads) so the per-iteration AllReduces see
    compute skew only, not core-launch skew.
"""

import os
import sys

import numpy as np

for _p in ("/opt/trn_rl_repo", "/root/.axon_site/_ro/trn_rl_repo"):
    if os.path.isdir(_p) and _p not in sys.path:
        sys.path.insert(0, _p)

import concourse.bass as bass  # noqa: E402
from concourse import bacc  # noqa: E402
import concourse.tile as tile  # noqa: E402
from concourse import mybir  # noqa: E402
from concourse import bass_utils  # noqa: E402

B, R, O, D, C = 128, 2048, 16, 16, 16
NCORES = 8
RLOC = R // NCORES  # 256
G = 4  # generation blocks, contiguous r ranges, d-bands at 32g
RB = RLOC // G  # 64 r's per block
HB = RB // 2  # 32 r's per PSUM bank
RCH = 16  # r chunk size in routing phase
NCH = RLOC // RCH  # 16
CPB = RB // RCH  # chunks per gen block
ROUTING_ITERS = 3
F16 = mybir.dt.float16
F32 = mybir.dt.float32
EXP = mybir.ActivationFunctionType.Exp
SQUARE = mybir.ActivationFunctionType.Square

LAST_EXEC_NS = None
_NC_CACHE = {}


def _mm(nc, out, lhsT, rhs, start, stop, tile_position=None):
    """Matmul that relies on a previously issued ldweights() for its
    stationary (sets InstMatmult.ldweights=False)."""
    bi = nc.tensor.matmul(
        out, lhsT, rhs, start=start, stop=stop, tile_position=tile_position
    )
    bi.ins.ldweights = False
    return bi


def _s_chunk(nc, st, scr, spsum, ch, with_sq=True):
    """s[:, :, ch] = sum_o q*u for one r chunk; drain to s_full (f16);
    optionally square (ACT) + ns = sum_c s^2 (TensorE) into ns_ps."""
    rs = slice(ch * RCH, (ch + 1) * RCH)
    P = scr.tile([128, O, C, RCH], F16, tag="P", name=f"Ps{ch}")
    qb = st.q[:, :, rs].unsqueeze(2).broadcast_to([128, O, C, RCH])
    nc.vector.tensor_mul(P, st.u[:, :, :, rs], qb)
    s_ps = spsum.tile([128, C, RCH], F32, tag="s", name=f"sps{ch}")
    ali = s_ps.unsqueeze(1).broadcast_to([128, 2, C, RCH])
    for k in range(O // 2):
        _mm(nc, ali, st.eye, P[:, 2 * k : 2 * k + 2],
            start=(k == 0), stop=(k == O // 2 - 1))
    nc.scalar.copy(st.s_full[:, :, rs], s_ps)
    if with_sq:
        sq = scr.tile([128, C, RCH], F16, tag="sq", name=f"sq{ch}")
        nc.scalar.activation(sq, s_ps, SQUARE)
        nsali = st.ns_ps[:, rs].unsqueeze(1).broadcast_to([128, C, RCH])
        _mm(nc, nsali, st.eye, sq, start=True, stop=True)


def _squash_tail(nc, st, r0, rlen):
    """rtf = sqrt(ns)/(1+ns) over [r0, r0+rlen); v = s*rtf in place."""
    rs = slice(r0, r0 + rlen)
    nc.scalar.sqrt(st.rt[:, rs], st.ns_ps[:, rs])
    nc.vector.tensor_scalar_add(st.ns[:, rs], st.ns_ps[:, rs], 1.0)
    nc.vector.reciprocal(st.ns[:, rs], st.ns[:, rs])
    nc.vector.tensor_mul(st.rtf[:, rs], st.rt[:, rs], st.ns[:, rs])
    rb = st.rtf[:, rs].unsqueeze(1).broadcast_to([128, C, rlen])
    nc.vector.tensor_mul(st.s_full[:, :, rs], st.s_full[:, :, rs], rb)


def _a_chunk(nc, st, scr, apsum, ch, init):
    """a = sum_c u*v for one r chunk; q <- exp(a) (init) or q*exp(a);
    after the last chunk of each quarter, reduce that quarter into zlq."""
    rs = slice(ch * RCH, (ch + 1) * RCH)
    P2 = scr.tile([128, O, C, RCH], F16, tag="P", name=f"Pa{ch}")
    vb = st.s_full[:, :, rs].unsqueeze(1).broadcast_to([128, O, C, RCH])
    nc.vector.tensor_mul(P2, st.u[:, :, :, rs], vb)
    a_ps = apsum.tile([128, O, RCH], F32, tag="a", name=f"aps{ch}")
    ali = a_ps.unsqueeze(1).broadcast_to([128, 2, O, RCH])
    for k in range(C // 2):
        rhs = P2[:, :, 2 * k : 2 * k + 2].rearrange("p o c r -> p c o r")
        _mm(nc, ali, st.eye, rhs,
            start=(k == 0), stop=(k == C // 2 - 1))
    if init:
        nc.scalar.activation(st.q[:, :, rs], a_ps, EXP)
    else:
        e = scr.tile([128, O, RCH], F16, tag="e", name=f"e{ch}")
        nc.scalar.activation(e, a_ps, EXP)
        nc.vector.tensor_mul(st.q[:, :, rs], st.q[:, :, rs], e)
    if ch % 4 == 3:
        _zl_quarter(nc, st, ch // 4)


def _zl_quarter(nc, st, qt):
    """zlq[:, :, qt] = sum over this quarter's 64 r's of q (f16 tree)."""
    lvl = st.q[:, :, qt * 64 : (qt + 1) * 64]
    nc.vector.tensor_add(st.z32, lvl[:, :, :32], lvl[:, :, 32:])
    nc.vector.tensor_add(st.z16, st.z32[:, :, :16], st.z32[:, :, 16:])
    nc.vector.tensor_reduce(
        st.zlq[:, :, qt : qt + 1], st.z16,
        axis=mybir.AxisListType.X, op=mybir.AluOpType.add,
    )


def _cc_issue(nc, st, dramp, it):
    """AllReduce the softmax denominator across cores."""
    nc.vector.tensor_reduce(
        st.zl, st.zlq, axis=mybir.AxisListType.X, op=mybir.AluOpType.add
    )
    cc_in = dramp.tile([128, O], F32, name=f"cc_in{it}")
    cc_out = dramp.tile([128, O], F32, name=f"cc_out{it}")
    nc.gpsimd.dma_start(out=cc_in, in_=st.zl)
    nc.gpsimd.collective_compute(
        "AllReduce",
        mybir.AluOpType.add,
        replica_groups=[list(range(NCORES))],
        ins=[cc_in.opt()],
        outs=[cc_out.opt()],
    )
    nc.gpsimd.dma_start(out=st.zg, in_=cc_out)


def _softmax_scale(nc, st):
    """q <- q / Z in place (per-o tensor_scalar so the DVE runs in 4x
    single-src mode)."""
    nc.vector.reciprocal(st.zg, st.zg)
    for o in range(O):
        nc.vector.tensor_scalar_mul(
            st.q[:, o], st.q[:, o], st.zg[:, o : o + 1]
        )


class _St:
    pass


def _body(tc, xt_ap, w_ap, wbar_ap, eye_ap, out_ap):
    nc = tc.nc
    st = _St()
    with (
        tc.tile_pool(name="const", bufs=1) as constp,
        tc.tile_pool(name="upool", bufs=1) as upool,
        tc.tile_pool(name="state", bufs=1) as stp,
        tc.tile_pool(name="scr", bufs=2) as scr,
        tc.tile_pool(name="apsum", bufs=1, space="PSUM") as apsum,
        tc.tile_pool(name="ccdram", bufs=2, space="DRAM") as dramp,
    ):
        st.xt16 = constp.tile([128, B], F16)
        st.eye = constp.tile([128, 128], F16)
        st.u = upool.tile([128, O, C, RLOC], F16)
        st.s_full = stp.tile([128, C, RLOC], F16)  # s, then v in place
        st.q = stp.tile([128, O, RLOC], F16)  # running softmax numerator
        st.ns = stp.tile([128, RLOC], F32)
        st.rt = stp.tile([128, RLOC], F32)
        st.rtf = stp.tile([128, RLOC], F16)
        st.zl = stp.tile([128, O], F32)
        st.zlA = stp.tile([128, O], F32)
        st.zlB = stp.tile([128, O], F32)
        st.zgA = stp.tile([128, O], F32)
        st.zgB = stp.tile([128, O], F32)
        st.zg = stp.tile([128, O], F32)
        st.zgf = stp.tile([128, O], F16)
        st.z32 = stp.tile([128, O, 32], F16)
        st.z16 = stp.tile([128, O, 16], F16)
        st.zlq = stp.tile([128, O, 4], F32)
        st.ccback = stp.tile([128, O], F32)
        st.ns_ps = apsum.tile([128, RLOC], F32, tag="ns", name="ns_ps")

        for g in range(G):
            nc.gpsimd.dma_start(out=st.xt16[32 * g : 32 * g + D, :], in_=xt_ap)

        # ---- generation: u = x@W, s0 = x@Wbar; iter-0 work one block behind
        with (
            tc.tile_pool(name="wpool", bufs=1) as wpool,
            tc.tile_pool(name="gpsum", bufs=3, space="PSUM") as gpsum,
        ):
            wt = wpool.tile([128, O, 2, C, HB], F16)
            wbt = wpool.tile([128, C, RB], F16)
            for g in range(G):
                nc.gpsimd.dma_start(
                    out=wbt[32 * g : 32 * g + D], in_=wbar_ap[g]
                )
            nc.gpsimd.dma_start(out=st.eye, in_=eye_ap)
            for g in (0, 1):
                for j in range(4):
                    nc.gpsimd.dma_start(
                        out=wt[32 * g : 32 * g + D, 4 * j : 4 * j + 4],
                        in_=w_ap[g, :, 4 * j : 4 * j + 4],
                    )
            # Warmup collective doubles as a cross-core barrier: reading
            # its output on the DMA queue gates the later weight loads,
            # so cores align here (with block-0/1 compute available to
            # hide the wait) instead of skewing the iteration AllReduces.
            nc.vector.memset(st.zl, 0.0)
            ccw_in = dramp.tile([128, O], F32, name="ccw_in")
            ccw_out = dramp.tile([128, O], F32, name="ccw_out")
            nc.gpsimd.dma_start(out=ccw_in, in_=st.zl)
            nc.gpsimd.collective_compute(
                "AllReduce",
                mybir.AluOpType.add,
                replica_groups=[list(range(NCORES))],
                ins=[ccw_in.opt()],
                outs=[ccw_out.opt()],
            )
            nc.gpsimd.dma_start(out=st.ccback, in_=ccw_out)
            for g in (2, 3):
                for j in range(4):
                    nc.gpsimd.dma_start(
                        out=wt[32 * g : 32 * g + D, 4 * j : 4 * j + 4],
                        in_=w_ap[g, :, 4 * j : 4 * j + 4],
                    )

            sqbs = {}

            def gen_s0_all():
                """All four band stationaries coexist (disjoint PE rows):
                load once, then all s0 matmuls loadless."""
                for g in range(G):
                    nc.tensor.ldweights(
                        st.xt16[32 * g : 32 * g + D, :],
                        tile_position=(32 * g, 0),
                    )
                for g in range(G):
                    band = st.xt16[32 * g : 32 * g + D, :]
                    rs = slice(g * RB, (g + 1) * RB)
                    s0 = gpsum.tile(
                        [128, 2, C, HB], F32, tag="u", name=f"s0_{g}"
                    )
                    for k in range(2):
                        _mm(nc, s0[:, k], band,
                            wbt[32 * g : 32 * g + D, :, k * HB : (k + 1) * HB],
                            start=True, stop=True,
                            tile_position=(32 * g, 0))
                    dst = st.s_full[:, :, rs].rearrange(
                        "p c (k r) -> p k c r", k=2
                    )
                    nc.scalar.copy(dst, s0)
                    sqb = scr.tile(
                        [128, 2, C, HB], F16, tag="sqb", name=f"sqb{g}"
                    )
                    sqbs[g] = sqb
                    nc.scalar.activation(sqb, s0, SQUARE)

            def squash0_all():
                """ldweights(eye) + ns matmuls + squash for all blocks."""
                nc.tensor.ldweights(st.eye)
                for g in range(G):
                    sqb = sqbs[g]
                    for k in range(2):
                        rsk = slice(g * RB + k * HB, g * RB + (k + 1) * HB)
                        nsali = st.ns_ps[:, rsk].unsqueeze(1).broadcast_to(
                            [128, C, HB]
                        )
                        _mm(nc, nsali, st.eye, sqb[:, k],
                            start=True, stop=True)
                for g in range(G):
                    _squash_tail(nc, st, g * RB, RB)

            def gen_u(g):
                """ldweights(xt band) + 32 u matmuls, loadless."""
                band = st.xt16[32 * g : 32 * g + D, :]
                tp = (32 * g, 0)
                rs = slice(g * RB, (g + 1) * RB)
                nc.tensor.ldweights(band, tile_position=tp)
                for o in range(O):
                    ps = gpsum.tile(
                        [128, 2, C, HB], F32, tag="u", name=f"ups{g}_{o}"
                    )
                    for k in range(2):
                        _mm(nc, ps[:, k], band, wt[32 * g : 32 * g + D, o, k],
                            start=True, stop=True, tile_position=tp)
                    dst = st.u[:, o, :, rs].rearrange(
                        "p c (k r) -> p k c r", k=2
                    )
                    if o % 4 == 3:
                        nc.vector.tensor_copy(dst, ps)
                    else:
                        nc.scalar.copy(dst, ps)

            def iter0_block(g):
                """ldweights(eye) + agreement chunks for block g."""
                nc.tensor.ldweights(st.eye)
                for ch in range(g * CPB, (g + 1) * CPB):
                    _a_chunk(nc, st, scr, apsum, ch, init=True)
                if g == G - 1:
                    _cc_issue(nc, st, dramp, 1)

            gen_s0_all()
            squash0_all()
            for g in range(G):
                gen_u(g)
                if g >= 1:
                    iter0_block(g - 1)
            iter0_block(G - 1)

        # ---------------- routing iterations 1..2 ----------------
        # (the PE still holds the identity from the last iter0_block)
        with tc.tile_pool(name="spsum", bufs=2, space="PSUM") as spsum:
            for it in range(1, ROUTING_ITERS):
                _softmax_scale(nc, st)
                if it < ROUTING_ITERS - 1:
                    # quarter-pipelined: squash each quarter as its s
                    # chunks land so agreement work starts early
                    rq4 = RLOC // 4
                    for qt in range(4):
                        for ch in range(qt * 4, (qt + 1) * 4):
                            _s_chunk(nc, st, scr, spsum, ch)
                        _squash_tail(nc, st, qt * rq4, rq4)
                        for ch in range(qt * 4, (qt + 1) * 4):
                            _a_chunk(nc, st, scr, apsum, ch, init=False)
                    _cc_issue(nc, st, dramp, 2)
                else:
                    # final: stream v out per quarter
                    rq4 = RLOC // 4
                    for qt in range(4):
                        for ch in range(qt * 4, (qt + 1) * 4):
                            _s_chunk(nc, st, scr, spsum, ch)
                        r0 = qt * rq4
                        _squash_tail(nc, st, r0, rq4)
                        nc.gpsimd.dma_start(
                            out=out_ap[:, :, r0 : r0 + rq4],
                            in_=st.s_full[:, :, r0 : r0 + rq4],
                        )


def _build_nc():
    nc = bacc.Bacc(
        "TRN2",
        target_bir_lowering=False,
        debug=False,
        enable_asserts=False,
        num_devices=NCORES,
    )
    xt_d = nc.dram_tensor("xt", [D, B], F32, kind="ExternalInput")
    w_d = nc.dram_tensor("w", [G, D, O, 2, C, HB], F16, kind="ExternalInput")
    wbar_d = nc.dram_tensor("wbar", [G, D, C, RB], F16, kind="ExternalInput")
    eye_d = nc.dram_tensor("eye", [128, 128], F16, kind="ExternalInput")
    out_d = nc.dram_tensor("out", [B, C, RLOC], F32, kind="ExternalOutput")

    with tile.TileContext(nc) as tc:
        _body(tc, xt_d.ap(), w_d.ap(), wbar_d.ap(), eye_d.ap(), out_d.ap())
    nc.compile()
    return nc


def _prep_inputs(x, route_weights):
    xt = np.ascontiguousarray(x.reshape(B, D).T.astype(np.float32))  # [D, B]
    w0 = np.asarray(route_weights).reshape(R, O, D, C)
    eye = np.eye(128, dtype=np.float16)
    in_maps = []
    for i in range(NCORES):
        ws = w0[i * RLOC : (i + 1) * RLOC]  # (RLOC, O, D, C)
        wg = ws.reshape(G, 2, HB, O, D, C)
        wprep = np.ascontiguousarray(
            wg.transpose(0, 4, 3, 1, 5, 2).astype(np.float16)
        )  # [G, D, O, 2, C, HB]
        wbar = (ws.sum(axis=1) / R).reshape(G, RB, D, C)
        wbprep = np.ascontiguousarray(
            wbar.transpose(0, 2, 3, 1).astype(np.float16)
        )  # [G, D, C, RB]
        in_maps.append({"xt": xt, "w": wprep, "wbar": wbprep, "eye": eye})
    return in_maps


def kernel(x, route_weights, trace=False):
    global LAST_EXEC_NS
    x = np.asarray(x, dtype=np.float32)
    route_weights = np.asarray(route_weights, dtype=np.float32)

    if "nc" not in _NC_CACHE:
        _NC_CACHE["nc"] = _build_nc()
    nc = _NC_CACHE["nc"]

    in_maps = _prep_inputs(x, route_weights)
    res = bass_utils.run_bass_kernel_spmd(
        nc, in_maps, core_ids=list(range(NCORES)), trace=trace
    )
    LAST_EXEC_NS = res.exec_time_ns

    shards = []
    for i in range(NCORES):
        o = res.results[i]["out"]  # [B, C, RLOC]
        shards.append(np.transpose(o, (0, 2, 1)))  # [B, RLOC, C]
    return np.concatenate(shards, axis=1).astype(np.float32)  # (B, R, C)


# revision 25
# speedup vs baseline: 1.0167x; 1.0167x over previous
"""DigitCapsules dynamic-routing kernel for 8 TRN2 NeuronCores.

Strategy (hardcoded for B=128, R=2048, O=16, D=16, C=16, 3 routing iters):
  - Shard R across the 8 cores (256 routes/core); x replicated.
  - u_hat = x @ W on TensorE (K=16 matmuls packed 4x via row tile_position),
    kept SBUF-resident as f16 [b=128 partitions, (o, c, r)], r innermost.
  - Stationaries are loaded once per run with standalone ldweights();
    the matmuls set InstMatmult.ldweights=False (verified on HW), so a
    matmul costs only its column stream.
  - All routing reductions run on TensorE: an identity-stationary matmul
    whose output AP is broadcast (stride 0) over the reduced dim makes PSUM
    accumulate the slices per element (verified on HW), so
      s = sum_o c*u, a = sum_c u*v, ns = sum_c s^2
    cost one PSUM-bank-sized matmul group streaming the product tile. DVE
    only does the elementwise products (f16, 2x mode); ScalarE drains PSUM
    and computes exp/square/sqrt (activation tables are preloaded once).
  - Iteration 0 uses uniform c_ij: s0 = x @ Wbar with Wbar = sum_o W / R
    precomputed on host.
  - Softmax state is multiplicative: q <- q * exp(a_psum); softmax over
    global R only needs the denominator AllReduce (8KB) per iteration;
    normalization is scale-invariant so q is rescaled in place.
  - The warmup AllReduce doubles as a cross-core barrier (its output DMA
    gates the later weight loads) so the per-iteration AllReduces see
    compute skew only, not core-launch skew.
"""

import os
import sys

import numpy as np

for _p in ("/opt/trn_rl_repo", "/root/.axon_site/_ro/trn_rl_repo"):
    if os.path.isdir(_p) and _p not in sys.path:
        sys.path.insert(0, _p)

import concourse.bass as bass  # noqa: E402
from concourse import bacc  # noqa: E402
import concourse.tile as tile  # noqa: E402
from concourse import mybir  # noqa: E402
from concourse import bass_utils  # noqa: E402

B, R, O, D, C = 128, 2048, 16, 16, 16
NCORES = 8
RLOC = R // NCORES  # 256
G = 4  # generation blocks, contiguous r ranges, d-bands at 32g
RB = RLOC // G  # 64 r's per block
HB = RB // 2  # 32 r's per PSUM bank
RCH = 16  # r chunk size in routing phase
NCH = RLOC // RCH  # 16
CPB = RB // RCH  # chunks per gen block
ROUTING_ITERS = 3
F16 = mybir.dt.float16
F32 = mybir.dt.float32
EXP = mybir.ActivationFunctionType.Exp
SQUARE = mybir.ActivationFunctionType.Square

LAST_EXEC_NS = None
_NC_CACHE = {}


def _mm(nc, out, lhsT, rhs, start, stop, tile_position=None):
    """Matmul that relies on a previously issued ldweights() for its
    stationary (sets InstMatmult.ldweights=False)."""
    bi = nc.tensor.matmul(
        out, lhsT, rhs, start=start, stop=stop, tile_position=tile_position
    )
    bi.ins.ldweights = False
    return bi


def _s_chunk(nc, st, scr, spsum, ch, with_sq=True):
    """s[:, :, ch] = sum_o q*u for one r chunk; drain to s_full (f16);
    optionally square (ACT) + ns = sum_c s^2 (TensorE) into ns_ps."""
    rs = slice(ch * RCH, (ch + 1) * RCH)
    P = scr.tile([128, O, C, RCH], F16, tag="P", name=f"Ps{ch}")
    qb = st.q[:, :, rs].unsqueeze(2).broadcast_to([128, O, C, RCH])
    nc.vector.tensor_mul(P, st.u[:, :, :, rs], qb)
    s_ps = spsum.tile([128, C, RCH], F32, tag="s", name=f"sps{ch}")
    ali = s_ps.unsqueeze(1).broadcast_to([128, 2, C, RCH])
    for k in range(O // 2):
        _mm(nc, ali, st.eye, P[:, 2 * k : 2 * k + 2],
            start=(k == 0), stop=(k == O // 2 - 1))
    nc.scalar.copy(st.s_full[:, :, rs], s_ps)
    if with_sq:
        sq = scr.tile([128, C, RCH], F16, tag="sq", name=f"sq{ch}")
        nc.scalar.activation(sq, s_ps, SQUARE)
        nsali = st.ns_ps[:, rs].unsqueeze(1).broadcast_to([128, C, RCH])
        _mm(nc, nsali, st.eye, sq, start=True, stop=True)


def _squash_tail(nc, st, r0, rlen):
    """rtf = sqrt(ns)/(1+ns) over [r0, r0+rlen); v = s*rtf in place."""
    rs = slice(r0, r0 + rlen)
    nc.scalar.sqrt(st.rt[:, rs], st.ns_ps[:, rs])
    nc.vector.tensor_scalar_add(st.ns[:, rs], st.ns_ps[:, rs], 1.0)
    nc.vector.reciprocal(st.ns[:, rs], st.ns[:, rs])
    nc.vector.tensor_mul(st.rtf[:, rs], st.rt[:, rs], st.ns[:, rs])
    rb = st.rtf[:, rs].unsqueeze(1).broadcast_to([128, C, rlen])
    nc.vector.tensor_mul(st.s_full[:, :, rs], st.s_full[:, :, rs], rb)


def _a_chunk(nc, st, scr, apsum, ch, init):
    """a = sum_c u*v for one r chunk; q <- exp(a) (init) or q*exp(a);
    after the last chunk of each quarter, reduce that quarter into zlq."""
    rs = slice(ch * RCH, (ch + 1) * RCH)
    P2 = scr.tile([128, O, C, RCH], F16, tag="P", name=f"Pa{ch}")
    vb = st.s_full[:, :, rs].unsqueeze(1).broadcast_to([128, O, C, RCH])
    nc.vector.tensor_mul(P2, st.u[:, :, :, rs], vb)
    a_ps = apsum.tile([128, O, RCH], F32, tag="a", name=f"aps{ch}")
    ali = a_ps.unsqueeze(1).broadcast_to([128, 2, O, RCH])
    for k in range(C // 2):
        rhs = P2[:, :, 2 * k : 2 * k + 2].rearrange("p o c r -> p c o r")
        _mm(nc, ali, st.eye, rhs,
            start=(k == 0), stop=(k == C // 2 - 1))
    if init:
        nc.scalar.activation(st.q[:, :, rs], a_ps, EXP)
    else:
        e = scr.tile([128, O, RCH], F16, tag="e", name=f"e{ch}")
        nc.scalar.activation(e, a_ps, EXP)
        nc.vector.tensor_mul(st.q[:, :, rs], st.q[:, :, rs], e)
    if ch % 4 == 3:
        _zl_quarter(nc, st, ch // 4)


def _zl_quarter(nc, st, qt):
    """zlq[:, :, qt] = sum over this quarter's 64 r's of q (f16 tree)."""
    lvl = st.q[:, :, qt * 64 : (qt + 1) * 64]
    nc.vector.tensor_add(st.z32, lvl[:, :, :32], lvl[:, :, 32:])
    nc.vector.tensor_add(st.z16, st.z32[:, :, :16], st.z32[:, :, 16:])
    nc.vector.tensor_reduce(
        st.zlq[:, :, qt : qt + 1], st.z16,
        axis=mybir.AxisListType.X, op=mybir.AluOpType.add,
    )


def _cc_issue(nc, st, dramp, it):
    """AllReduce the softmax denominator across cores."""
    nc.vector.tensor_reduce(
        st.zl, st.zlq, axis=mybir.AxisListType.X, op=mybir.AluOpType.add
    )
    cc_in = dramp.tile([128, O], F32, name=f"cc_in{it}")
    cc_out = dramp.tile([128, O], F32, name=f"cc_out{it}")
    nc.gpsimd.dma_start(out=cc_in, in_=st.zl)
    nc.gpsimd.collective_compute(
        "AllReduce",
        mybir.AluOpType.add,
        replica_groups=[list(range(NCORES))],
        ins=[cc_in.opt()],
        outs=[cc_out.opt()],
    )
    nc.gpsimd.dma_start(out=st.zg, in_=cc_out)


def _softmax_scale(nc, st):
    """q <- q / Z in place (per-o tensor_scalar so the DVE runs in 4x
    single-src mode)."""
    nc.vector.reciprocal(st.zg, st.zg)
    for o in range(O):
        nc.vector.tensor_scalar_mul(
            st.q[:, o], st.q[:, o], st.zg[:, o : o + 1]
        )


class _St:
    pass


def _body(tc, xt_ap, w_ap, wbar_ap, eye_ap, out_ap):
    nc = tc.nc
    st = _St()
    with (
        tc.tile_pool(name="const", bufs=1) as constp,
        tc.tile_pool(name="upool", bufs=1) as upool,
        tc.tile_pool(name="state", bufs=1) as stp,
        tc.tile_pool(name="scr", bufs=2) as scr,
        tc.tile_pool(name="apsum", bufs=1, space="PSUM") as apsum,
        tc.tile_pool(name="ccdram", bufs=2, space="DRAM") as dramp,
    ):
        st.xt16 = constp.tile([128, B], F16)
        st.eye = constp.tile([128, 128], F16)
        st.u = upool.tile([128, O, C, RLOC], F16)
        st.s_full = stp.tile([128, C, RLOC], F16)  # s, then v in place
        st.q = stp.tile([128, O, RLOC], F16)  # running softmax numerator
        st.ns = stp.tile([128, RLOC], F32)
        st.rt = stp.tile([128, RLOC], F32)
        st.rtf = stp.tile([128, RLOC], F16)
        st.zl = stp.tile([128, O], F32)
        st.zlA = stp.tile([128, O], F32)
        st.zlB = stp.tile([128, O], F32)
        st.zgA = stp.tile([128, O], F32)
        st.zgB = stp.tile([128, O], F32)
        st.zg = stp.tile([128, O], F32)
        st.zgf = stp.tile([128, O], F16)
        st.z32 = stp.tile([128, O, 32], F16)
        st.z16 = stp.tile([128, O, 16], F16)
        st.zlq = stp.tile([128, O, 4], F32)
        st.ccback = stp.tile([128, O], F32)
        st.ns_ps = apsum.tile([128, RLOC], F32, tag="ns", name="ns_ps")

        for g in range(G):
            nc.gpsimd.dma_start(out=st.xt16[32 * g : 32 * g + D, :], in_=xt_ap)

        # ---- generation: u = x@W, s0 = x@Wbar; iter-0 work one block behind
        with (
            tc.tile_pool(name="wpool", bufs=1) as wpool,
            tc.tile_pool(name="gpsum", bufs=3, space="PSUM") as gpsum,
        ):
            wt = wpool.tile([128, O, 2, C, HB], F16)
            wbt = wpool.tile([128, C, RB], F16)
            for g in range(G):
                nc.gpsimd.dma_start(
                    out=wbt[32 * g : 32 * g + D], in_=wbar_ap[g]
                )
            nc.gpsimd.dma_start(out=st.eye, in_=eye_ap)
            for g in (0, 1):
                for j in range(4):
                    nc.gpsimd.dma_start(
                        out=wt[32 * g : 32 * g + D, 4 * j : 4 * j + 4],
                        in_=w_ap[g, :, 4 * j : 4 * j + 4],
                    )
            # Warmup collective doubles as a cross-core barrier: reading
            # its output on the DMA queue gates the later weight loads,
            # so cores align here (with block-0/1 compute available to
            # hide the wait) instead of skewing the iteration AllReduces.
            nc.vector.memset(st.zl, 0.0)
            ccw_in = dramp.tile([128, O], F32, name="ccw_in")
            ccw_out = dramp.tile([128, O], F32, name="ccw_out")
            nc.gpsimd.dma_start(out=ccw_in, in_=st.zl)
            nc.gpsimd.collective_compute(
                "AllReduce",
                mybir.AluOpType.add,
                replica_groups=[list(range(NCORES))],
                ins=[ccw_in.opt()],
                outs=[ccw_out.opt()],
            )
            nc.gpsimd.dma_start(out=st.ccback, in_=ccw_out)
            for g in (2, 3):
                for j in range(4):
                    nc.gpsimd.dma_start(
                        out=wt[32 * g : 32 * g + D, 4 * j : 4 * j + 4],
                        in_=w_ap[g, :, 4 * j : 4 * j + 4],
                    )

            sqbs = {}

            def gen_s0_all():
                """All four band stationaries coexist (disjoint PE rows):
                load once, then all s0 matmuls loadless."""
                for g in range(G):
                    nc.tensor.ldweights(
                        st.xt16[32 * g : 32 * g + D, :],
                        tile_position=(32 * g, 0),
                    )
                for g in range(G):
                    band = st.xt16[32 * g : 32 * g + D, :]
                    rs = slice(g * RB, (g + 1) * RB)
                    s0 = gpsum.tile(
                        [128, 2, C, HB], F32, tag="u", name=f"s0_{g}"
                    )
                    for k in range(2):
                        _mm(nc, s0[:, k], band,
                            wbt[32 * g : 32 * g + D, :, k * HB : (k + 1) * HB],
                            start=True, stop=True,
                            tile_position=(32 * g, 0))
                    dst = st.s_full[:, :, rs].rearrange(
                        "p c (k r) -> p k c r", k=2
                    )
                    nc.scalar.copy(dst, s0)
                    sqb = scr.tile(
                        [128, 2, C, HB], F16, tag="sqb", name=f"sqb{g}"
                    )
                    sqbs[g] = sqb
                    nc.scalar.activation(sqb, s0, SQUARE)

            def squash0_all():
                """ldweights(eye) + ns matmuls + squash for all blocks."""
                nc.tensor.ldweights(st.eye)
                for g in range(G):
                    sqb = sqbs[g]
                    for k in range(2):
                        rsk = slice(g * RB + k * HB, g * RB + (k + 1) * HB)
                        nsali = st.ns_ps[:, rsk].unsqueeze(1).broadcast_to(
                            [128, C, HB]
                        )
                        _mm(nc, nsali, st.eye, sqb[:, k],
                            start=True, stop=True)
                for g in range(G):
                    _squash_tail(nc, st, g * RB, RB)

            def gen_u(g):
                """ldweights(xt band) + 32 u matmuls, loadless."""
                band = st.xt16[32 * g : 32 * g + D, :]
                tp = (32 * g, 0)
                rs = slice(g * RB, (g + 1) * RB)
                nc.tensor.ldweights(band, tile_position=tp)
                for o in range(O):
                    ps = gpsum.tile(
                        [128, 2, C, HB], F32, tag="u", name=f"ups{g}_{o}"
                    )
                    for k in range(2):
                        _mm(nc, ps[:, k], band, wt[32 * g : 32 * g + D, o, k],
                            start=True, stop=True, tile_position=tp)
                    dst = st.u[:, o, :, rs].rearrange(
                        "p c (k r) -> p k c r", k=2
                    )
                    if o % 4 == 3:
                        nc.vector.tensor_copy(dst, ps)
                    else:
                        nc.scalar.copy(dst, ps)

            def iter0_block(g):
                """ldweights(eye) + agreement chunks for block g."""
                nc.tensor.ldweights(st.eye)
                for ch in range(g * CPB, (g + 1) * CPB):
                    _a_chunk(nc, st, scr, apsum, ch, init=True)
                if g == G - 1:
                    _cc_issue(nc, st, dramp, 1)

            gen_s0_all()
            squash0_all()
            for g in range(G):
                gen_u(g)
                if g >= 1:
                    iter0_block(g - 1)
            iter0_block(G - 1)

        # ---------------- routing iterations 1..2 ----------------
        # (the PE still holds the identity from the last iter0_block)
        with tc.tile_pool(name="spsum", bufs=2, space="PSUM") as spsum:
            for it in range(1, ROUTING_ITERS):
                _softmax_scale(nc, st)
                if it < ROUTING_ITERS - 1:
                    # quarter-pipelined one deep: quarter qt's squash and
                    # agreement run while quarter qt+1's s chunks stream,
                    # so the DVE queue never head-of-line blocks on the
                    # ns matmul chain
                    rq4 = RLOC // 4
                    for qt in range(4):
                        for ch in range(qt * 4, (qt + 1) * 4):
                            _s_chunk(nc, st, scr, spsum, ch)
                        if qt >= 1:
                            _squash_tail(nc, st, (qt - 1) * rq4, rq4)
                            for ch in range((qt - 1) * 4, qt * 4):
                                _a_chunk(nc, st, scr, apsum, ch, init=False)
                    _squash_tail(nc, st, 3 * rq4, rq4)
                    for ch in range(12, 16):
                        _a_chunk(nc, st, scr, apsum, ch, init=False)
                    _cc_issue(nc, st, dramp, 2)
                else:
                    # final: stream v out per quarter, one quarter behind
                    rq4 = RLOC // 4

                    def _finish_qt(qt):
                        r0 = qt * rq4
                        _squash_tail(nc, st, r0, rq4)
                        nc.gpsimd.dma_start(
                            out=out_ap[:, :, r0 : r0 + rq4],
                            in_=st.s_full[:, :, r0 : r0 + rq4],
                        )

                    for qt in range(4):
                        for ch in range(qt * 4, (qt + 1) * 4):
                            _s_chunk(nc, st, scr, spsum, ch)
                        if qt >= 1:
                            _finish_qt(qt - 1)
                    _finish_qt(3)


def _build_nc():
    nc = bacc.Bacc(
        "TRN2",
        target_bir_lowering=False,
        debug=False,
        enable_asserts=False,
        num_devices=NCORES,
    )
    xt_d = nc.dram_tensor("xt", [D, B], F32, kind="ExternalInput")
    w_d = nc.dram_tensor("w", [G, D, O, 2, C, HB], F16, kind="ExternalInput")
    wbar_d = nc.dram_tensor("wbar", [G, D, C, RB], F16, kind="ExternalInput")
    eye_d = nc.dram_tensor("eye", [128, 128], F16, kind="ExternalInput")
    out_d = nc.dram_tensor("out", [B, C, RLOC], F32, kind="ExternalOutput")

    with tile.TileContext(nc) as tc:
        _body(tc, xt_d.ap(), w_d.ap(), wbar_d.ap(), eye_d.ap(), out_d.ap())
    nc.compile()
    return nc


def _prep_inputs(x, route_weights):
    xt = np.ascontiguousarray(x.reshape(B, D).T.astype(np.float32))  # [D, B]
    w0 = np.asarray(route_weights).reshape(R, O, D, C)
    eye = np.eye(128, dtype=np.float16)
    in_maps = []
    for i in range(NCORES):
        ws = w0[i * RLOC : (i + 1) * RLOC]  # (RLOC, O, D, C)
        wg = ws.reshape(G, 2, HB, O, D, C)
        wprep = np.ascontiguousarray(
            wg.transpose(0, 4, 3, 1, 5, 2).astype(np.float16)
        )  # [G, D, O, 2, C, HB]
        wbar = (ws.sum(axis=1) / R).reshape(G, RB, D, C)
        wbprep = np.ascontiguousarray(
            wbar.transpose(0, 2, 3, 1).astype(np.float16)
        )  # [G, D, C, RB]
        in_maps.append({"xt": xt, "w": wprep, "wbar": wbprep, "eye": eye})
    return in_maps


def kernel(x, route_weights, trace=False):
    global LAST_EXEC_NS
    x = np.asarray(x, dtype=np.float32)
    route_weights = np.asarray(route_weights, dtype=np.float32)

    if "nc" not in _NC_CACHE:
        _NC_CACHE["nc"] = _build_nc()
    nc = _NC_CACHE["nc"]

    in_maps = _prep_inputs(x, route_weights)
    res = bass_utils.run_bass_kernel_spmd(
        nc, in_maps, core_ids=list(range(NCORES)), trace=trace
    )
    LAST_EXEC_NS = res.exec_time_ns

    shards = []
    for i in range(NCORES):
        o = res.results[i]["out"]  # [B, C, RLOC]
        shards.append(np.transpose(o, (0, 2, 1)))  # [B, RLOC, C]
    return np.concatenate(shards, axis=1).astype(np.float32)  # (B, R, C)
